# revision 22
# baseline (speedup 1.0000x reference)
"""MoE FeedForward (dense 8-expert, top-2 gate) TRN2 Bass kernel — v3 bf16.

Sharding: 8 shards = (batch b in 0..3) x (H-half in {top, bottom}).
Each NeuronCore computes all 8 experts + gate + top-2 combine for its
32-row spatial slab. Input shards carry a 1-row halo (depthwise conv);
gather on host is pure concatenation.

v3 (vs v1 baseline):
  - Up-projection in bf16 (FWL-eligible weight loads, vs fp32r which
    loads weights slowly) over a bf16 xs (x*inv) + augmented rows
    t1=-mu*inv (weights s1) and hmask (weights c1).
  - Single x load (gate logits + stats read the same f32r x tiles).
  - Unpadded h tiles [128, 34*64]: depthwise edge handling via
    column-restricted out/rhs APs (first op = full-coverage dx=1 tap);
    no pad-column memsets.
  - GELU in 1024-col instructions.
  - fp16 output accumulators (DVE mult by gate weight, Pool add);
    fp16 output DMA, host casts to fp32.
  - Tap engines per chunk tunable: PE diag-matmul / MIX (DVE TS+TT
    tree) / AD (ACT muls + DVE/Pool add tree).
"""
import numpy as np
import ml_dtypes

DIM, MULT, E, TOPK = 192, 4, 8, 2
INNER = DIM * MULT            # 768
B, H, W = 4, 64, 64
EPS = 1e-5
ROWS = 34                     # 32 + 2 halo
NPOS = ROWS * W               # 2176
NOUT = 32 * W                 # 2048
NCHUNK = INNER // 128         # 6

# tap engine per chunk: 'PE' | 'MIX' | 'AD'
TAPS = {0: "PE", 1: "PE", 2: "PE", 3: "MIX", 4: "MIX", 5: "AD"}
# h psum->sbuf copy engine per chunk
HCOPY = {0: "ACT", 1: "DVE", 2: "ACT", 3: "DVE", 4: "ACT", 5: "ACT"}

BF16 = ml_dtypes.bfloat16

UPT = [(0, 512), (512, 512), (1024, 512), (1536, 512), (2048, 128)]

_CACHE = {}


def _build_nc():
    import concourse.bacc as bacc
    import concourse.tile as tile
    import concourse.bass as bass
    from concourse import mybir

    F32 = mybir.dt.float32
    F32R = mybir.dt.float32r
    BF = mybir.dt.bfloat16
    F16 = mybir.dt.float16
    AF = mybir.ActivationFunctionType
    OP = mybir.AluOpType

    nc = bacc.Bacc("TRN2", target_bir_lowering=False)

    # ---- dram tensors ----
    dx0 = nc.dram_tensor("x0", [128, NPOS], F32R, kind="ExternalInput")
    dx1 = nc.dram_tensor("x1", [64, NPOS], F32R, kind="ExternalInput")
    dhm = nc.dram_tensor("hmask", [1, NPOS], BF, kind="ExternalInput")
    dwg0 = nc.dram_tensor("wg0", [128, 8], F32, kind="ExternalInput")
    dwg1 = nc.dram_tensor("wg1", [64, 8], F32, kind="ExternalInput")
    dx0f = nc.dram_tensor("x0f", [128, NPOS], F32, kind="ExternalInput")
    dx1f = nc.dram_tensor("x1f", [64, NPOS], F32, kind="ExternalInput")
    dbg = nc.dram_tensor("bg", [8, 1], F32, kind="ExternalInput")
    dones = nc.dram_tensor("ones", [128, 1], F32R, kind="ExternalInput")
    dw1a0 = nc.dram_tensor("w1a0", [E, 128, INNER], BF, kind="ExternalInput")
    dw1a1 = nc.dram_tensor("w1a1", [E, 66, INNER], BF, kind="ExternalInput")
    dw2t = nc.dram_tensor("w2t", [E, 128, NCHUNK * DIM], BF, kind="ExternalInput")
    ddiag = nc.dram_tensor("diag", [E, 128, NCHUNK * 9 * 128], BF,
                           kind="ExternalInput")
    ddwc = nc.dram_tensor("dwc", [E, 128, NCHUNK * 9], F32, kind="ExternalInput")
    dbdw = nc.dram_tensor("bdw", [E, 128, NCHUNK], F32, kind="ExternalInput")
    dc2 = nc.dram_tensor("c2s", [8, DIM], F32R, kind="ExternalInput")
    dident = nc.dram_tensor("ident", [128, 128], F32, kind="ExternalInput")
    dout = nc.dram_tensor("out", [DIM, NOUT], F32, kind="ExternalOutput")
    dinvs = nc.dram_tensor("invscratch", [1, NPOS], F32, kind="Internal")
    dwalls = nc.dram_tensor("wallscratch", [8, NOUT], BF, kind="Internal")

    NT_ALL = [(0, 512), (512, 512), (1024, 512), (1536, 320), (1856, 320)]
    PE_CH = [mc for mc in range(NCHUNK) if TAPS[mc] == "PE"]

    with tile.TileContext(nc) as tc:
        with tc.tile_pool(name="persist", bufs=1) as pp, \
             tc.tile_pool(name="acc", bufs=1) as accp:
            x0 = pp.tile([128, NPOS], F32R)
            x1 = pp.tile([64, NPOS], F32R)
            xs0 = pp.tile([128, NPOS], BF)
            xs1 = pp.tile([66, NPOS], BF)
            ident = pp.tile([128, 128], F32)
            wg0 = pp.tile([128, 8], F32)
            wg1 = pp.tile([64, 8], F32)
            bg = pp.tile([8, 1], F32)
            ones = pp.tile([128, 1], F32R)
            w_all = pp.tile([8, NOUT], F32R)
            out0 = accp.tile([128, NOUT], F32)
            out1 = accp.tile([64, NOUT], F32)

            nc.sync.dma_start(out=x0, in_=dx0[:, :])
            nc.sync.dma_start(out=x1, in_=dx1[:, :])
            nc.gpsimd.dma_start(out=xs1[65:66, :], in_=dhm[:, :])
            nc.sync.dma_start(out=ident, in_=dident[:, :])
            nc.sync.dma_start(out=wg0, in_=dwg0[:, :])
            nc.sync.dma_start(out=wg1, in_=dwg1[:, :])
            nc.sync.dma_start(out=bg, in_=dbg[:, :])
            nc.sync.dma_start(out=ones, in_=dones[:, :])

            # ---------------- stage 0 ----------------
            with tc.tile_pool(name="s0sb", bufs=2) as s0sb, \
                 tc.tile_pool(name="s0ps", bufs=1, space="PSUM") as s0ps, \
                 tc.tile_pool(name="s0row", bufs=1) as s0row:
                Lsb = s0row.tile([8, NOUT], F32)
                S1row = s0row.tile([1, NPOS], F32)
                S2row = s0row.tile([1, NPOS], F32)
                x0f = s0sb.tile([128, NPOS], F32, tag="x0f", bufs=1)
                x1f = s0sb.tile([64, NPOS], F32, tag="x1f", bufs=1)
                nc.sync.dma_start(out=x0f, in_=dx0f[:, :])
                nc.sync.dma_start(out=x1f, in_=dx1f[:, :])

                for i in range(4):
                    o = 64 + i * 512
                    pl = s0ps.tile([8, 512], F32, tag="pl")
                    nc.tensor.matmul(pl, wg0[:], x0f[:, o:o + 512],
                                     start=True, stop=False)
                    nc.tensor.matmul(pl, wg1[:], x1f[:, o:o + 512],
                                     start=False, stop=True)
                    nc.vector.tensor_scalar(out=Lsb[:, i * 512:(i + 1) * 512],
                                            in0=pl, scalar1=bg[:, :], scalar2=None,
                                            op0=OP.add)

                for (o, n) in NT_ALL:
                    q0 = s0sb.tile([128, 512], F32R, tag="q0")
                    q1 = s0sb.tile([64, 512], F32R, tag="q1")
                    nc.scalar.activation(q0[:, 0:n], x0[:, o:o + n], AF.Square)
                    nc.scalar.activation(q1[:, 0:n], x1[:, o:o + n], AF.Square)
                    psS1 = s0ps.tile([1, 512], F32, tag="psS1")
                    nc.tensor.matmul(psS1[:, 0:n], ones[:], x0[:, o:o + n],
                                     start=True, stop=False)
                    nc.tensor.matmul(psS1[:, 0:n], ones[0:64, :], x1[:, o:o + n],
                                     start=False, stop=True)
                    nc.vector.tensor_copy(S1row[:, o:o + n], psS1[:, 0:n])
                    psS2 = s0ps.tile([1, 512], F32, tag="psS2")
                    nc.tensor.matmul(psS2[:, 0:n], ones[:], q0[:, 0:n],
                                     start=True, stop=False)
                    nc.tensor.matmul(psS2[:, 0:n], ones[0:64, :], q1[:, 0:n],
                                     start=False, stop=True)
                    nc.vector.tensor_copy(S2row[:, o:o + n], psS2[:, 0:n])

                sbeps = s0row.tile([1, 1], F32)
                nc.vector.memset(sbeps, EPS)
                inv = s0row.tile([1, NPOS], F32)
                musq = s0row.tile([1, NPOS], F32)
                nc.scalar.activation(musq, S1row[:, :], AF.Square, scale=1.0 / DIM)
                v1 = s0row.tile([1, NPOS], F32)
                nc.vector.scalar_tensor_tensor(out=v1, in0=S2row[:, :],
                                               scalar=1.0 / DIM, in1=musq,
                                               op0=OP.mult, op1=OP.subtract)
                sd = s0row.tile([1, NPOS], F32)
                nc.scalar.activation(sd, v1, AF.Sqrt, bias=sbeps[:, :], scale=1.0)
                nc.vector.reciprocal_approx_fast(inv, sd)
                nc.vector.scalar_tensor_tensor(out=xs1[64:65, :], in0=S1row[:, :],
                                               scalar=1.0 / DIM, in1=inv,
                                               op0=OP.mult, op1=OP.mult)
                nc.sync.dma_start(out=dinvs[:, :], in_=inv)
                ivap = dinvs[0:1, :]
                inv_src = bass.AP(tensor=ivap.tensor, offset=ivap.offset,
                                  ap=[[0, 128]] + ivap.ap[1:])
                inv_b = s0row.tile([128, NPOS], F32)
                nc.gpsimd.dma_start(out=inv_b, in_=inv_src)
                for (o, n) in NT_ALL:
                    nc.vector.tensor_mul(xs0[:, o:o + n], x0[:, o:o + n],
                                         inv_b[:, o:o + n])
                    nc.vector.tensor_mul(xs1[0:64, o:o + n], x1[:, o:o + n],
                                         inv_b[0:64, o:o + n])

                # ---- top-2 gate in transposed layout ----
                LT = s0row.tile([128, 128], F32)
                for c in range(16):
                    pt = s0ps.tile([128, 8], F32, tag="pt")
                    nc.tensor.transpose(pt, Lsb[:, c * 128:(c + 1) * 128],
                                        ident[0:8, 0:8])
                    nc.vector.tensor_copy(LT[:, c * 8:(c + 1) * 8], pt)
                LTv = LT[:, :].rearrange("p (c e) -> p c e", e=8)
                M1 = s0row.tile([128, 16], F32)
                nc.vector.tensor_reduce(M1, LTv, axis=mybir.AxisListType.X, op=OP.max)

                def bc8(t):
                    a = t[:, :]
                    return bass.AP(tensor=a.tensor, offset=a.offset,
                                   ap=a.ap + [[0, 8]])

                LR = s0row.tile([128, 128], F32)
                nc.vector.tensor_sub(LR[:, :].rearrange("p (c e) -> p c e", e=8),
                                     LTv, bc8(M1))
                EQ = s0row.tile([128, 128], F32)
                nc.vector.tensor_scalar(out=EQ, in0=LR, scalar1=0.0, scalar2=None,
                                        op0=OP.is_equal)
                TMP = s0row.tile([128, 128], F32)
                nc.vector.scalar_tensor_tensor(out=TMP, in0=EQ, scalar=-1e30,
                                               in1=LR, op0=OP.mult, op1=OP.add)
                M2 = s0row.tile([128, 16], F32)
                nc.vector.tensor_reduce(M2, TMP[:, :].rearrange("p (c e) -> p c e", e=8),
                                        axis=mybir.AxisListType.X, op=OP.max)
                EX = s0row.tile([128, 128], F32)
                nc.scalar.activation(EX, LR, AF.Exp)
                ED = s0row.tile([128, 16], F32)
                nc.scalar.activation(ED, M2, AF.Exp)
                DEN = s0row.tile([128, 16], F32)
                nc.vector.tensor_scalar(out=DEN, in0=ED, scalar1=1.0, scalar2=None,
                                        op0=OP.add)
                RC = s0row.tile([128, 16], F32)
                nc.vector.reciprocal_approx_fast(RC, DEN)
                KEEP = s0row.tile([128, 128], F32)
                nc.vector.tensor_tensor(out=KEEP[:, :].rearrange("p (c e) -> p c e", e=8),
                                        in0=LR[:, :].rearrange("p (c e) -> p c e", e=8),
                                        in1=bc8(M2), op=OP.is_ge)
                WT = s0row.tile([128, 128], F32)
                nc.vector.tensor_mul(WT, EX, KEEP)
                nc.vector.tensor_mul(WT[:, :].rearrange("p (c e) -> p c e", e=8),
                                     WT[:, :].rearrange("p (c e) -> p c e", e=8),
                                     bc8(RC))
                for c in range(16):
                    pw = s0ps.tile([8, 128], F32, tag="pw")
                    nc.tensor.transpose(pw, WT[:, c * 8:(c + 1) * 8], ident[:, :])
                    nc.vector.tensor_copy(w_all[:, c * 128:(c + 1) * 128], pw)
                wbf = s0row.tile([8, NOUT], BF)
                nc.vector.tensor_copy(wbf, w_all)
                nc.sync.dma_start(out=dwalls[:, :], in_=wbf)

                c2sb = s0sb.tile([8, DIM], F32R, tag="c2")
                nc.sync.dma_start(out=c2sb, in_=dc2[:, :])
                for i in range(4):
                    o = i * 512
                    pd0 = s0ps.tile([128, 512], F32, tag="pd0i")
                    nc.tensor.matmul(pd0, c2sb[:, 0:128], w_all[:, o:o + 512],
                                     start=True, stop=True)
                    nc.vector.tensor_copy(out0[:, o:o + 512], pd0)
                    pd1 = s0ps.tile([64, 512], F32, tag="pd1i")
                    nc.tensor.matmul(pd1, c2sb[:, 128:192], w_all[:, o:o + 512],
                                     start=True, stop=True)
                    nc.vector.tensor_copy(out1[:, o:o + 512], pd1)

            # ---------------- expert loop ----------------
            with tc.tile_pool(name="wts", bufs=2) as wts, \
                 tc.tile_pool(name="hp", bufs=2) as hp, \
                 tc.tile_pool(name="gw", bufs=2) as gwp, \
                 tc.tile_pool(name="tap", bufs=2) as tapp, \
                 tc.tile_pool(name="cmb", bufs=2) as cmbp, \
                 tc.tile_pool(name="psw", bufs=2, space="PSUM") as psw, \
                 tc.tile_pool(name="psd0", bufs=2, space="PSUM") as psd0p, \
                 tc.tile_pool(name="psd1", bufs=2, space="PSUM") as psd1p:
                for e in range(E):
                    W1A0 = wts.tile([128, INNER], BF, tag="w1a0")
                    W1A1 = wts.tile([66, INNER], BF, tag="w1a1")
                    W2T = wts.tile([128, NCHUNK * DIM], BF, tag="w2t")
                    BDW = wts.tile([128, NCHUNK], F32, tag="bdw")
                    W_B = wts.tile([128, NOUT], BF, tag="wb")
                    nc.sync.dma_start(out=W1A0, in_=dw1a0[e, :, :])
                    nc.sync.dma_start(out=W1A1, in_=dw1a1[e, :, :])
                    nc.sync.dma_start(out=W2T, in_=dw2t[e, :, :])
                    nc.sync.dma_start(out=BDW, in_=dbdw[e, :, :])
                    if PE_CH:
                        DIAG = wts.tile([128, len(PE_CH) * 9 * 128], BF,
                                        tag="diag")
                        for ci, mc in enumerate(PE_CH):
                            nc.sync.dma_start(
                                out=DIAG[:, ci * 9 * 128:(ci + 1) * 9 * 128],
                                in_=ddiag[e, :, mc * 9 * 128:(mc + 1) * 9 * 128])
                    if len(PE_CH) < NCHUNK:
                        DWC = wts.tile([128, NCHUNK * 9], F32, tag="dwc")
                        nc.sync.dma_start(out=DWC, in_=ddwc[e, :, :])
                    wsrc = dwalls[e:e + 1, :]
                    nc.gpsimd.dma_start(
                        out=W_B,
                        in_=bass.AP(tensor=wsrc.tensor, offset=wsrc.offset,
                                    ap=[[0, 128]] + wsrc.ap[1:]))

                    PW = W + 2
                    hpt = []
                    for mc in range(NCHUNK):
                        if TAPS[mc] == "PE":
                            ht = hp.tile([128, NPOS], BF, tag=f"h{mc}")
                        else:
                            ht = hp.tile([128, ROWS * PW], BF, tag=f"h{mc}")
                            hview = ht[:, :].rearrange("p (r c) -> p r c", c=PW)
                            nc.gpsimd.memset(hview[:, :, 0:1], 0.0)
                            nc.gpsimd.memset(hview[:, :, PW - 1:PW], 0.0)
                        hpt.append(ht)
                    g8 = gwp.tile([128, NCHUNK * NOUT], BF, tag="g8")

                    # ---- up-projection + depthwise taps + gelu, interleaved ----
                    # tap k at (dy,dx)=(k//3,k%3): out(y,x)+=dw_k*h(y+dy-1,x+dx-1)
                    # out rows 1..33; width edges via restricted columns.
                    def colspec(dx):
                        if dx == 0:
                            return W - 1, 1      # ncol, out col base
                        if dx == 1:
                            return W, 0
                        return W - 1, 0

                    for mc in (3, 4, 5, 0, 1, 2):
                        for (o, n) in UPT:
                            ph = psw.tile([128, 512], F32, tag="ph")
                            nc.tensor.matmul(ph[:, 0:n],
                                             W1A0[:, mc * 128:(mc + 1) * 128],
                                             xs0[:, o:o + n],
                                             start=True, stop=False)
                            nc.tensor.matmul(ph[:, 0:n],
                                             W1A1[:, mc * 128:(mc + 1) * 128],
                                             xs1[:, o:o + n],
                                             start=False, stop=True)
                            if TAPS[mc] == "PE":
                                dst = hpt[mc][:, o:o + n]
                            else:
                                r0c = o // W
                                nrc = n // W
                                dst = hpt[mc][:, :].rearrange(
                                    "p (r c) -> p r c", c=PW)[
                                    :, r0c:r0c + nrc, 1:W + 1]
                            if HCOPY[mc] == "ACT":
                                nc.scalar.copy(dst, ph[:, 0:n])
                            else:
                                nc.vector.tensor_copy(dst, ph[:, 0:n])

                    for mc in range(NCHUNK):
                        hap = hpt[mc][:, :]
                        eng = TAPS[mc]
                        if eng == "PE":
                            for d in range(4):
                                pt = psw.tile([128, 512], F32, tag="ptap")
                                ptv = pt[:, :].rearrange("p (r c) -> p r c", c=W)
                                r0 = 1 + 8 * d
                                order = [1, 0, 2, 3, 4, 5, 6, 7, 8]
                                for si, k in enumerate(order):
                                    dy, dx = k // 3, k % 3
                                    ncol, ob = colspec(dx)
                                    ci = PE_CH.index(mc)
                                    dg = DIAG[:, (ci * 9 + k) * 128:
                                              (ci * 9 + k + 1) * 128]
                                    base = hap.offset + (r0 + dy - 1) * W \
                                        + (ob + dx - 1)
                                    rhs = bass.AP(
                                        tensor=hap.tensor, offset=base,
                                        ap=[hap.ap[0], [W, 8], [1, ncol]])
                                    nc.tensor.matmul(
                                        ptv[:, :, ob:ob + ncol], dg, rhs,
                                        start=(si == 0), stop=(si == 8))
                                nc.scalar.activation(
                                    g8[:, mc * NOUT + d * 512:
                                       mc * NOUT + (d + 1) * 512],
                                    pt, AF.Gelu, bias=BDW[:, mc:mc + 1],
                                    scale=1.0)
                        else:
                            hv66 = hap.rearrange("p (r c) -> p r c", c=PW)
                            for ti in range(2):
                                r0 = 1 + 16 * ti

                                def sc(k):
                                    return DWC[:, mc * 9 + k:mc * 9 + k + 1]

                                def view(k):
                                    dy, dx = k // 3, k % 3
                                    return hv66[:, r0 + dy - 1:r0 + dy + 15,
                                                dx:dx + W]

                                acc = tapp.tile([128, 1024], BF, tag="ta")
                                accv = acc[:, :].rearrange("p (r c) -> p r c", c=W)
                                hb = tapp.tile([128, 1024], BF, tag="tb")
                                hbv = hb[:, :].rearrange("p (r c) -> p r c", c=W)
                                if eng in ("MIX", "MIXP"):
                                    nc.vector.tensor_scalar(
                                        out=accv, in0=view(0), scalar1=sc(0),
                                        scalar2=None, op0=OP.mult)
                                    nc.vector.tensor_scalar(
                                        out=hbv, in0=view(4), scalar1=sc(4),
                                        scalar2=None, op0=OP.mult)
                                    for k in (1, 2, 3, 5, 6, 7, 8):
                                        tm = tapp.tile([128, 1024], BF, tag="tm")
                                        tmv = tm[:, :].rearrange(
                                            "p (r c) -> p r c", c=W)
                                        nc.vector.tensor_scalar(
                                            out=tmv, in0=view(k), scalar1=sc(k),
                                            scalar2=None, op0=OP.mult)
                                        dst = acc if k < 4 else hb
                                        if eng == "MIX":
                                            nc.vector.tensor_add(dst, dst, tm)
                                        else:
                                            nc.gpsimd.tensor_add(dst, dst, tm)
                                    if eng == "MIX":
                                        nc.vector.tensor_add(acc, acc, hb)
                                    else:
                                        nc.gpsimd.tensor_add(acc, acc, hb)
                                else:  # AD: ACT muls; adds DVE + Pool trees
                                    nc.scalar.mul(accv, view(0), sc(0))
                                    nc.scalar.mul(hbv, view(4), sc(4))
                                    for k in (1, 2, 3, 5, 6, 7, 8):
                                        tm = tapp.tile([128, 1024], BF, tag="tm")
                                        tmv = tm[:, :].rearrange(
                                            "p (r c) -> p r c", c=W)
                                        nc.scalar.mul(tmv, view(k), sc(k))
                                        if k < 4:
                                            nc.vector.tensor_add(acc, acc, tm)
                                        else:
                                            nc.vector.tensor_add(hb, hb, tm)
                                    nc.vector.tensor_add(acc, acc, hb)
                                nc.scalar.activation(
                                    g8[:, mc * NOUT + ti * 1024:
                                       mc * NOUT + (ti + 1) * 1024],
                                    acc, AF.Gelu, bias=BDW[:, mc:mc + 1],
                                    scale=1.0)

                    # ---- down-projection + combine, per dtile ----
                    for dt in range(4):
                        o = dt * 512
                        pd0 = psd0p.tile([128, 512], F32, tag="pd0")
                        pd1 = psd1p.tile([64, 512], F32, tag="pd1")
                        for kc in range(NCHUNK):
                            nc.tensor.matmul(pd0, W2T[:, kc * DIM:kc * DIM + 128],
                                             g8[:, kc * NOUT + o:kc * NOUT + o + 512],
                                             start=(kc == 0),
                                             stop=(kc == NCHUNK - 1))
                        for kc in range(NCHUNK):
                            nc.tensor.matmul(pd1,
                                             W2T[:, kc * DIM + 128:(kc + 1) * DIM],
                                             g8[:, kc * NOUT + o:kc * NOUT + o + 512],
                                             start=(kc == 0),
                                             stop=(kc == NCHUNK - 1))
                        t0 = cmbp.tile([128, 512], F32, tag="t0")
                        t1_ = cmbp.tile([64, 512], F32, tag="t1")
                        nc.vector.tensor_mul(t0, pd0, W_B[:, o:o + 512])
                        nc.vector.tensor_mul(t1_, pd1, W_B[0:64, o:o + 512])
                        nc.gpsimd.tensor_add(out0[:, o:o + 512],
                                             out0[:, o:o + 512], t0)
                        nc.gpsimd.tensor_add(out1[:, o:o + 512],
                                             out1[:, o:o + 512], t1_)

            nc.sync.dma_start(out=dout[0:128, :], in_=out0)
            nc.sync.dma_start(out=dout[128:192, :], in_=out1)
    nc.compile()
    return nc


def _host_prep(x, ln_g, ln_b, w1, b1, dw, bdw, w2, b2, wg, bg):
    f = np.float32
    shared = {}
    W1g = w1 * ln_g[:, None, :]                        # (E, INNER, DIM)
    s1 = -W1g.sum(axis=2)                              # (E, INNER), negated
    c1 = np.einsum('eic,ec->ei', w1, ln_b) + b1        # (E, INNER)
    shared["w1a0"] = np.ascontiguousarray(
        np.transpose(W1g[:, :, 0:128], (0, 2, 1))).astype(BF16)
    w1a1 = np.concatenate([
        np.transpose(W1g[:, :, 128:192], (0, 2, 1)),
        s1[:, None, :], c1[:, None, :]], axis=1)
    shared["w1a1"] = np.ascontiguousarray(w1a1).astype(BF16)
    w2t = np.transpose(w2, (0, 2, 1)).reshape(E, NCHUNK, 128, DIM)
    shared["w2t"] = np.ascontiguousarray(
        np.transpose(w2t, (0, 2, 1, 3)).reshape(E, 128, NCHUNK * DIM)
    ).astype(BF16)
    dwf = dw[:, :, 0]                                  # (E, INNER, 3, 3)
    eye = np.eye(128, dtype=f)
    diag = np.zeros((E, 128, NCHUNK, 9, 128), f)
    for mc in range(NCHUNK):
        for k in range(9):
            dy, dx = k // 3, k % 3
            col = dwf[:, mc * 128:(mc + 1) * 128, dy, dx]
            diag[:, :, mc, k, :] = col[:, :, None] * eye[None, :, :]
    shared["diag"] = diag.astype(BF16).reshape(E, 128, NCHUNK * 9 * 128)
    dwc = np.zeros((E, 128, NCHUNK * 9), f)
    for mc in range(NCHUNK):
        for k in range(9):
            dy, dx = k // 3, k % 3
            dwc[:, :, mc * 9 + k] = dwf[:, mc * 128:(mc + 1) * 128, dy, dx]
    shared["dwc"] = dwc
    shared["bdw"] = np.ascontiguousarray(
        bdw.reshape(E, NCHUNK, 128).transpose(0, 2, 1)).astype(f)
    shared["c2s"] = np.ascontiguousarray(b2).astype(f)
    shared["wg0"] = np.ascontiguousarray(wg.T[0:128]).astype(f)
    shared["wg1"] = np.ascontiguousarray(wg.T[128:192]).astype(f)
    shared["bg"] = bg.reshape(8, 1).astype(f)
    shared["ident"] = np.eye(128, dtype=f)
    shared["ones"] = np.ones((128, 1), f)

    in_maps = []
    xp = np.zeros((B, DIM, H + 2, W), f)
    xp[:, :, 1:H + 1, :] = x
    for core in range(8):
        b, half = core // 2, core % 2
        r0 = half * 32
        xs = xp[b, :, r0:r0 + ROWS, :]
        hm = np.ones((1, ROWS, W), f)
        if half == 0:
            hm[:, 0, :] = 0
        else:
            hm[:, ROWS - 1, :] = 0
        m = dict(shared)
        m["x0"] = np.ascontiguousarray(xs[0:128].reshape(128, NPOS))
        m["x1"] = np.ascontiguousarray(xs[128:192].reshape(64, NPOS))
        m["x0f"] = m["x0"]
        m["x1f"] = m["x1"]
        m["hmask"] = hm.reshape(1, NPOS).astype(BF16)
        in_maps.append(m)
    return in_maps


def _run(inputs, trace=False):
    from concourse.bass_utils import run_bass_kernel_spmd
    if "nc" not in _CACHE:
        _CACHE["nc"] = _build_nc()
    nc = _CACHE["nc"]
    in_maps = _host_prep(**inputs)
    res = run_bass_kernel_spmd(nc, in_maps, core_ids=list(range(8)), trace=trace)
    out = np.empty((B, DIM, H, W), np.float32)
    for core in range(8):
        b, half = core // 2, core % 2
        out[b, :, half * 32:(half + 1) * 32, :] = \
            np.asarray(res.results[core]["out"]).reshape(DIM, 32, W)
    return out, res


def kernel(**inputs) -> np.ndarray:
    inputs = {k: np.asarray(v, dtype=np.float32) for k, v in inputs.items()}
    out, _ = _run(inputs, trace=False)
    return out


# revision 23
# speedup vs baseline: 1.0163x; 1.0163x over previous
"""MoE FeedForward (dense 8-expert, top-2 gate) TRN2 Bass kernel — v3 bf16.

Sharding: 8 shards = (batch b in 0..3) x (H-half in {top, bottom}).
Each NeuronCore computes all 8 experts + gate + top-2 combine for its
32-row spatial slab. Input shards carry a 1-row halo (depthwise conv);
gather on host is pure concatenation.

v3 (vs v1 baseline):
  - Up-projection in bf16 (FWL-eligible weight loads, vs fp32r which
    loads weights slowly) over a bf16 xs (x*inv) + augmented rows
    t1=-mu*inv (weights s1) and hmask (weights c1).
  - Single x load (gate logits + stats read the same f32r x tiles).
  - Unpadded h tiles [128, 34*64]: depthwise edge handling via
    column-restricted out/rhs APs (first op = full-coverage dx=1 tap);
    no pad-column memsets.
  - GELU in 1024-col instructions.
  - fp16 output accumulators (DVE mult by gate weight, Pool add);
    fp16 output DMA, host casts to fp32.
  - Tap engines per chunk tunable: PE diag-matmul / MIX (DVE TS+TT
    tree) / AD (ACT muls + DVE/Pool add tree).
"""
import numpy as np
import ml_dtypes

DIM, MULT, E, TOPK = 192, 4, 8, 2
INNER = DIM * MULT            # 768
B, H, W = 4, 64, 64
EPS = 1e-5
ROWS = 34                     # 32 + 2 halo
NPOS = ROWS * W               # 2176
NOUT = 32 * W                 # 2048
NCHUNK = INNER // 128         # 6

# tap engine per chunk: 'PE' | 'MIX' | 'AD'
TAPS = {0: "PE", 1: "PE", 2: "PE", 3: "MIX", 4: "MIX", 5: "AD"}
# h psum->sbuf copy engine per chunk
HCOPY = {0: "ACT", 1: "DVE", 2: "ACT", 3: "DVE", 4: "ACT", 5: "ACT"}

BF16 = ml_dtypes.bfloat16

UPT = [(0, 512), (512, 512), (1024, 512), (1536, 512), (2048, 128)]

_CACHE = {}


def _build_nc():
    import concourse.bacc as bacc
    import concourse.tile as tile
    import concourse.bass as bass
    from concourse import mybir

    F32 = mybir.dt.float32
    F32R = mybir.dt.float32r
    BF = mybir.dt.bfloat16
    F16 = mybir.dt.float16
    AF = mybir.ActivationFunctionType
    OP = mybir.AluOpType

    nc = bacc.Bacc("TRN2", target_bir_lowering=False)

    # ---- dram tensors ----
    dx0 = nc.dram_tensor("x0", [128, NPOS], F32R, kind="ExternalInput")
    dx1 = nc.dram_tensor("x1", [64, NPOS], F32R, kind="ExternalInput")
    dhm = nc.dram_tensor("hmask", [1, NPOS], BF, kind="ExternalInput")
    dwg0 = nc.dram_tensor("wg0", [128, 8], F32, kind="ExternalInput")
    dwg1 = nc.dram_tensor("wg1", [64, 8], F32, kind="ExternalInput")
    dx0f = nc.dram_tensor("x0f", [128, NPOS], F32, kind="ExternalInput")
    dx1f = nc.dram_tensor("x1f", [64, NPOS], F32, kind="ExternalInput")
    dbg = nc.dram_tensor("bg", [8, 1], F32, kind="ExternalInput")
    dones = nc.dram_tensor("ones", [128, 1], F32R, kind="ExternalInput")
    dw1a0 = nc.dram_tensor("w1a0", [E, 128, INNER], BF, kind="ExternalInput")
    dw1a1 = nc.dram_tensor("w1a1", [E, 66, INNER], BF, kind="ExternalInput")
    dw2t = nc.dram_tensor("w2t", [E, 128, NCHUNK * DIM], BF, kind="ExternalInput")
    ddiag = nc.dram_tensor("diag", [E, 128, NCHUNK * 9 * 128], BF,
                           kind="ExternalInput")
    ddwc = nc.dram_tensor("dwc", [E, 128, NCHUNK * 9], F32, kind="ExternalInput")
    dbdw = nc.dram_tensor("bdw", [E, 128, NCHUNK], F32, kind="ExternalInput")
    dc2 = nc.dram_tensor("c2s", [8, DIM], F32R, kind="ExternalInput")
    dident = nc.dram_tensor("ident", [128, 128], F32, kind="ExternalInput")
    dout = nc.dram_tensor("out", [DIM, NOUT], F32, kind="ExternalOutput")
    dinvs = nc.dram_tensor("invscratch", [1, NPOS], F32, kind="Internal")
    dwalls = nc.dram_tensor("wallscratch", [8, NOUT], BF, kind="Internal")

    NT_ALL = [(0, 512), (512, 512), (1024, 512), (1536, 320), (1856, 320)]
    PE_CH = [mc for mc in range(NCHUNK) if TAPS[mc] == "PE"]

    with tile.TileContext(nc) as tc:
        with tc.tile_pool(name="persist", bufs=1) as pp, \
             tc.tile_pool(name="acc", bufs=1) as accp:
            x0 = pp.tile([128, NPOS], F32R)
            x1 = pp.tile([64, NPOS], F32R)
            xs0 = pp.tile([128, NPOS], BF)
            xs1 = pp.tile([66, NPOS], BF)
            ident = pp.tile([128, 128], F32)
            wg0 = pp.tile([128, 8], F32)
            wg1 = pp.tile([64, 8], F32)
            bg = pp.tile([8, 1], F32)
            ones = pp.tile([128, 1], F32R)
            w_all = pp.tile([8, NOUT], F32R)
            out0 = accp.tile([128, NOUT], F32)
            out1 = accp.tile([64, NOUT], F32)

            nc.sync.dma_start(out=x0, in_=dx0[:, :])
            nc.sync.dma_start(out=x1, in_=dx1[:, :])
            nc.gpsimd.dma_start(out=xs1[65:66, :], in_=dhm[:, :])
            nc.sync.dma_start(out=ident, in_=dident[:, :])
            nc.sync.dma_start(out=wg0, in_=dwg0[:, :])
            nc.sync.dma_start(out=wg1, in_=dwg1[:, :])
            nc.sync.dma_start(out=bg, in_=dbg[:, :])
            nc.sync.dma_start(out=ones, in_=dones[:, :])

            # ---------------- stage 0 ----------------
            with tc.tile_pool(name="s0sb", bufs=2) as s0sb, \
                 tc.tile_pool(name="s0ps", bufs=1, space="PSUM") as s0ps, \
                 tc.tile_pool(name="s0row", bufs=1) as s0row:
                Lsb = s0row.tile([8, NOUT], F32)
                S1row = s0row.tile([1, NPOS], F32)
                S2row = s0row.tile([1, NPOS], F32)
                x0f = s0sb.tile([128, NPOS], F32, tag="x0f", bufs=1)
                x1f = s0sb.tile([64, NPOS], F32, tag="x1f", bufs=1)
                nc.sync.dma_start(out=x0f, in_=dx0f[:, :])
                nc.sync.dma_start(out=x1f, in_=dx1f[:, :])

                for i in range(4):
                    o = 64 + i * 512
                    pl = s0ps.tile([8, 512], F32, tag="pl")
                    nc.tensor.matmul(pl, wg0[:], x0f[:, o:o + 512],
                                     start=True, stop=False)
                    nc.tensor.matmul(pl, wg1[:], x1f[:, o:o + 512],
                                     start=False, stop=True)
                    nc.vector.tensor_scalar(out=Lsb[:, i * 512:(i + 1) * 512],
                                            in0=pl, scalar1=bg[:, :], scalar2=None,
                                            op0=OP.add)

                for (o, n) in NT_ALL:
                    q0 = s0sb.tile([128, 512], F32R, tag="q0")
                    q1 = s0sb.tile([64, 512], F32R, tag="q1")
                    nc.scalar.activation(q0[:, 0:n], x0[:, o:o + n], AF.Square)
                    nc.scalar.activation(q1[:, 0:n], x1[:, o:o + n], AF.Square)
                    psS1 = s0ps.tile([1, 512], F32, tag="psS1")
                    nc.tensor.matmul(psS1[:, 0:n], ones[:], x0[:, o:o + n],
                                     start=True, stop=False)
                    nc.tensor.matmul(psS1[:, 0:n], ones[0:64, :], x1[:, o:o + n],
                                     start=False, stop=True)
                    nc.vector.tensor_copy(S1row[:, o:o + n], psS1[:, 0:n])
                    psS2 = s0ps.tile([1, 512], F32, tag="psS2")
                    nc.tensor.matmul(psS2[:, 0:n], ones[:], q0[:, 0:n],
                                     start=True, stop=False)
                    nc.tensor.matmul(psS2[:, 0:n], ones[0:64, :], q1[:, 0:n],
                                     start=False, stop=True)
                    nc.vector.tensor_copy(S2row[:, o:o + n], psS2[:, 0:n])

                sbeps = s0row.tile([1, 1], F32)
                nc.vector.memset(sbeps, EPS)
                inv = s0row.tile([1, NPOS], F32)
                musq = s0row.tile([1, NPOS], F32)
                nc.scalar.activation(musq, S1row[:, :], AF.Square, scale=1.0 / DIM)
                v1 = s0row.tile([1, NPOS], F32)
                nc.vector.scalar_tensor_tensor(out=v1, in0=S2row[:, :],
                                               scalar=1.0 / DIM, in1=musq,
                                               op0=OP.mult, op1=OP.subtract)
                sd = s0row.tile([1, NPOS], F32)
                nc.scalar.activation(sd, v1, AF.Sqrt, bias=sbeps[:, :], scale=1.0)
                nc.vector.reciprocal_approx_fast(inv, sd)
                nc.vector.scalar_tensor_tensor(out=xs1[64:65, :], in0=S1row[:, :],
                                               scalar=1.0 / DIM, in1=inv,
                                               op0=OP.mult, op1=OP.mult)
                nc.sync.dma_start(out=dinvs[:, :], in_=inv)
                ivap = dinvs[0:1, :]
                inv_src = bass.AP(tensor=ivap.tensor, offset=ivap.offset,
                                  ap=[[0, 128]] + ivap.ap[1:])
                inv_b = s0row.tile([128, NPOS], F32)
                nc.gpsimd.dma_start(out=inv_b, in_=inv_src)
                for (o, n) in NT_ALL:
                    nc.vector.tensor_mul(xs0[:, o:o + n], x0[:, o:o + n],
                                         inv_b[:, o:o + n])
                    nc.vector.tensor_mul(xs1[0:64, o:o + n], x1[:, o:o + n],
                                         inv_b[0:64, o:o + n])

                # ---- top-2 gate in transposed layout ----
                LT = s0row.tile([128, 128], F32)
                for c in range(16):
                    pt = s0ps.tile([128, 8], F32, tag="pt")
                    nc.tensor.transpose(pt, Lsb[:, c * 128:(c + 1) * 128],
                                        ident[0:8, 0:8])
                    nc.vector.tensor_copy(LT[:, c * 8:(c + 1) * 8], pt)
                LTv = LT[:, :].rearrange("p (c e) -> p c e", e=8)
                M1 = s0row.tile([128, 16], F32)
                nc.vector.tensor_reduce(M1, LTv, axis=mybir.AxisListType.X, op=OP.max)

                def bc8(t):
                    a = t[:, :]
                    return bass.AP(tensor=a.tensor, offset=a.offset,
                                   ap=a.ap + [[0, 8]])

                LR = s0row.tile([128, 128], F32)
                nc.vector.tensor_sub(LR[:, :].rearrange("p (c e) -> p c e", e=8),
                                     LTv, bc8(M1))
                EQ = s0row.tile([128, 128], F32)
                nc.vector.tensor_scalar(out=EQ, in0=LR, scalar1=0.0, scalar2=None,
                                        op0=OP.is_equal)
                TMP = s0row.tile([128, 128], F32)
                nc.vector.scalar_tensor_tensor(out=TMP, in0=EQ, scalar=-1e30,
                                               in1=LR, op0=OP.mult, op1=OP.add)
                M2 = s0row.tile([128, 16], F32)
                nc.vector.tensor_reduce(M2, TMP[:, :].rearrange("p (c e) -> p c e", e=8),
                                        axis=mybir.AxisListType.X, op=OP.max)
                EX = s0row.tile([128, 128], F32)
                nc.scalar.activation(EX, LR, AF.Exp)
                ED = s0row.tile([128, 16], F32)
                nc.scalar.activation(ED, M2, AF.Exp)
                DEN = s0row.tile([128, 16], F32)
                nc.vector.tensor_scalar(out=DEN, in0=ED, scalar1=1.0, scalar2=None,
                                        op0=OP.add)
                RC = s0row.tile([128, 16], F32)
                nc.vector.reciprocal_approx_fast(RC, DEN)
                KEEP = s0row.tile([128, 128], F32)
                nc.vector.tensor_tensor(out=KEEP[:, :].rearrange("p (c e) -> p c e", e=8),
                                        in0=LR[:, :].rearrange("p (c e) -> p c e", e=8),
                                        in1=bc8(M2), op=OP.is_ge)
                WT = s0row.tile([128, 128], F32)
                nc.vector.tensor_mul(WT, EX, KEEP)
                nc.vector.tensor_mul(WT[:, :].rearrange("p (c e) -> p c e", e=8),
                                     WT[:, :].rearrange("p (c e) -> p c e", e=8),
                                     bc8(RC))
                for c in range(16):
                    pw = s0ps.tile([8, 128], F32, tag="pw")
                    nc.tensor.transpose(pw, WT[:, c * 8:(c + 1) * 8], ident[:, :])
                    nc.vector.tensor_copy(w_all[:, c * 128:(c + 1) * 128], pw)
                wbf = s0row.tile([8, NOUT], BF)
                nc.vector.tensor_copy(wbf, w_all)
                nc.sync.dma_start(out=dwalls[:, :], in_=wbf)

                c2sb = s0sb.tile([8, DIM], F32R, tag="c2")
                nc.sync.dma_start(out=c2sb, in_=dc2[:, :])
                for i in range(4):
                    o = i * 512
                    pd0 = s0ps.tile([128, 512], F32, tag="pd0i")
                    nc.tensor.matmul(pd0, c2sb[:, 0:128], w_all[:, o:o + 512],
                                     start=True, stop=True)
                    nc.vector.tensor_copy(out0[:, o:o + 512], pd0)
                    pd1 = s0ps.tile([64, 512], F32, tag="pd1i")
                    nc.tensor.matmul(pd1, c2sb[:, 128:192], w_all[:, o:o + 512],
                                     start=True, stop=True)
                    nc.vector.tensor_copy(out1[:, o:o + 512], pd1)

            # ---------------- expert loop ----------------
            with tc.tile_pool(name="wts", bufs=2) as wts, \
                 tc.tile_pool(name="hp", bufs=2) as hp, \
                 tc.tile_pool(name="gw", bufs=2) as gwp, \
                 tc.tile_pool(name="tap", bufs=2) as tapp, \
                 tc.tile_pool(name="cmb", bufs=2) as cmbp, \
                 tc.tile_pool(name="psw", bufs=2, space="PSUM") as psw, \
                 tc.tile_pool(name="psd0", bufs=2, space="PSUM") as psd0p, \
                 tc.tile_pool(name="psd1", bufs=2, space="PSUM") as psd1p:
                for e in range(E):
                    W1A0 = wts.tile([128, INNER], BF, tag="w1a0")
                    W1A1 = wts.tile([66, INNER], BF, tag="w1a1")
                    W2T = wts.tile([128, NCHUNK * DIM], BF, tag="w2t")
                    BDW = wts.tile([128, NCHUNK], F32, tag="bdw")
                    W_B = wts.tile([128, NOUT], BF, tag="wb")
                    nc.sync.dma_start(out=W1A0, in_=dw1a0[e, :, :])
                    nc.sync.dma_start(out=W1A1, in_=dw1a1[e, :, :])
                    nc.sync.dma_start(out=W2T, in_=dw2t[e, :, :])
                    nc.sync.dma_start(out=BDW, in_=dbdw[e, :, :])
                    if PE_CH:
                        DIAG = wts.tile([128, len(PE_CH) * 9 * 128], BF,
                                        tag="diag")
                        for ci, mc in enumerate(PE_CH):
                            nc.sync.dma_start(
                                out=DIAG[:, ci * 9 * 128:(ci + 1) * 9 * 128],
                                in_=ddiag[e, :, mc * 9 * 128:(mc + 1) * 9 * 128])
                    if len(PE_CH) < NCHUNK:
                        DWC = wts.tile([128, NCHUNK * 9], F32, tag="dwc")
                        nc.sync.dma_start(out=DWC, in_=ddwc[e, :, :])
                    wsrc = dwalls[e:e + 1, :]
                    nc.gpsimd.dma_start(
                        out=W_B,
                        in_=bass.AP(tensor=wsrc.tensor, offset=wsrc.offset,
                                    ap=[[0, 128]] + wsrc.ap[1:]))

                    PW = W + 2
                    hpt = []
                    for mc in range(NCHUNK):
                        if TAPS[mc] == "PE":
                            ht = hp.tile([128, NPOS], BF, tag=f"h{mc}")
                        else:
                            ht = hp.tile([128, ROWS * PW], BF, tag=f"h{mc}")
                            hview = ht[:, :].rearrange("p (r c) -> p r c", c=PW)
                            nc.gpsimd.memset(hview[:, :, 0:1], 0.0)
                            nc.gpsimd.memset(hview[:, :, PW - 1:PW], 0.0)
                        hpt.append(ht)
                    g8 = gwp.tile([128, NCHUNK * NOUT], BF, tag="g8")

                    # ---- up-projection + depthwise taps + gelu, interleaved ----
                    # tap k at (dy,dx)=(k//3,k%3): out(y,x)+=dw_k*h(y+dy-1,x+dx-1)
                    # out rows 1..33; width edges via restricted columns.
                    def colspec(dx):
                        if dx == 0:
                            return W - 1, 1      # ncol, out col base
                        if dx == 1:
                            return W, 0
                        return W - 1, 0

                    for mc in (3, 4, 5, 0, 1, 2):
                        for (o, n) in UPT:
                            ph = psw.tile([128, 512], F32, tag="ph")
                            nc.tensor.matmul(ph[:, 0:n],
                                             W1A0[:, mc * 128:(mc + 1) * 128],
                                             xs0[:, o:o + n],
                                             start=True, stop=False)
                            nc.tensor.matmul(ph[:, 0:n],
                                             W1A1[:, mc * 128:(mc + 1) * 128],
                                             xs1[:, o:o + n],
                                             start=False, stop=True)
                            if TAPS[mc] == "PE":
                                dst = hpt[mc][:, o:o + n]
                            else:
                                r0c = o // W
                                nrc = n // W
                                dst = hpt[mc][:, :].rearrange(
                                    "p (r c) -> p r c", c=PW)[
                                    :, r0c:r0c + nrc, 1:W + 1]
                            if HCOPY[mc] == "ACT":
                                nc.scalar.copy(dst, ph[:, 0:n])
                            else:
                                nc.vector.tensor_copy(dst, ph[:, 0:n])

                    for mc in range(NCHUNK):
                        hap = hpt[mc][:, :]
                        eng = TAPS[mc]
                        if eng == "PE":
                            for d in range(4):
                                pt = psw.tile([128, 512], F32, tag="ptap")
                                ptv = pt[:, :].rearrange("p (r c) -> p r c", c=W)
                                r0 = 1 + 8 * d
                                order = [1, 0, 2, 3, 4, 5, 6, 7, 8]
                                for si, k in enumerate(order):
                                    dy, dx = k // 3, k % 3
                                    ncol, ob = colspec(dx)
                                    ci = PE_CH.index(mc)
                                    dg = DIAG[:, (ci * 9 + k) * 128:
                                              (ci * 9 + k + 1) * 128]
                                    base = hap.offset + (r0 + dy - 1) * W \
                                        + (ob + dx - 1)
                                    rhs = bass.AP(
                                        tensor=hap.tensor, offset=base,
                                        ap=[hap.ap[0], [W, 8], [1, ncol]])
                                    nc.tensor.matmul(
                                        ptv[:, :, ob:ob + ncol], dg, rhs,
                                        start=(si == 0), stop=(si == 8))
                                nc.scalar.activation(
                                    g8[:, mc * NOUT + d * 512:
                                       mc * NOUT + (d + 1) * 512],
                                    pt, AF.Gelu, bias=BDW[:, mc:mc + 1],
                                    scale=1.0)
                        else:
                            hv66 = hap.rearrange("p (r c) -> p r c", c=PW)
                            for ti in range(2):
                                r0 = 1 + 16 * ti

                                def sc(k):
                                    return DWC[:, mc * 9 + k:mc * 9 + k + 1]

                                def view(k):
                                    dy, dx = k // 3, k % 3
                                    return hv66[:, r0 + dy - 1:r0 + dy + 15,
                                                dx:dx + W]

                                acc = tapp.tile([128, 1024], BF, tag="ta")
                                accv = acc[:, :].rearrange("p (r c) -> p r c", c=W)
                                hb = tapp.tile([128, 1024], BF, tag="tb")
                                hbv = hb[:, :].rearrange("p (r c) -> p r c", c=W)
                                if eng in ("MIX", "MIXP"):
                                    nc.vector.tensor_scalar(
                                        out=accv, in0=view(0), scalar1=sc(0),
                                        scalar2=None, op0=OP.mult)
                                    nc.vector.tensor_scalar(
                                        out=hbv, in0=view(4), scalar1=sc(4),
                                        scalar2=None, op0=OP.mult)
                                    for k in (1, 2, 3, 5, 6, 7, 8):
                                        tm = tapp.tile([128, 1024], BF, tag="tm")
                                        tmv = tm[:, :].rearrange(
                                            "p (r c) -> p r c", c=W)
                                        nc.vector.tensor_scalar(
                                            out=tmv, in0=view(k), scalar1=sc(k),
                                            scalar2=None, op0=OP.mult)
                                        dst = acc if k < 4 else hb
                                        if k in (5, 6):
                                            nc.gpsimd.tensor_add(dst, dst, tm)
                                        else:
                                            nc.vector.tensor_add(dst, dst, tm)
                                    nc.vector.tensor_add(acc, acc, hb)
                                else:  # AD: ACT muls; adds DVE + Pool trees
                                    nc.scalar.mul(accv, view(0), sc(0))
                                    nc.scalar.mul(hbv, view(4), sc(4))
                                    for k in (1, 2, 3, 5, 6, 7, 8):
                                        tm = tapp.tile([128, 1024], BF, tag="tm")
                                        tmv = tm[:, :].rearrange(
                                            "p (r c) -> p r c", c=W)
                                        nc.scalar.mul(tmv, view(k), sc(k))
                                        if k < 4:
                                            nc.vector.tensor_add(acc, acc, tm)
                                        elif k in (5, 6):
                                            nc.gpsimd.tensor_add(hb, hb, tm)
                                        else:
                                            nc.vector.tensor_add(hb, hb, tm)
                                    nc.vector.tensor_add(acc, acc, hb)
                                nc.scalar.activation(
                                    g8[:, mc * NOUT + ti * 1024:
                                       mc * NOUT + (ti + 1) * 1024],
                                    acc, AF.Gelu, bias=BDW[:, mc:mc + 1],
                                    scale=1.0)

                    # ---- down-projection + combine, per dtile ----
                    for dt in range(4):
                        o = dt * 512
                        pd0 = psd0p.tile([128, 512], F32, tag="pd0")
                        pd1 = psd1p.tile([64, 512], F32, tag="pd1")
                        for kc in range(NCHUNK):
                            nc.tensor.matmul(pd0, W2T[:, kc * DIM:kc * DIM + 128],
                                             g8[:, kc * NOUT + o:kc * NOUT + o + 512],
                                             start=(kc == 0),
                                             stop=(kc == NCHUNK - 1))
                        for kc in range(NCHUNK):
                            nc.tensor.matmul(pd1,
                                             W2T[:, kc * DIM + 128:(kc + 1) * DIM],
                                             g8[:, kc * NOUT + o:kc * NOUT + o + 512],
                                             start=(kc == 0),
                                             stop=(kc == NCHUNK - 1))
                        t0 = cmbp.tile([128, 512], F32, tag="t0")
                        t1_ = cmbp.tile([64, 512], F32, tag="t1")
                        nc.vector.tensor_mul(t0, pd0, W_B[:, o:o + 512])
                        nc.vector.tensor_mul(t1_, pd1, W_B[0:64, o:o + 512])
                        nc.gpsimd.tensor_add(out0[:, o:o + 512],
                                             out0[:, o:o + 512], t0)
                        nc.gpsimd.tensor_add(out1[:, o:o + 512],
                                             out1[:, o:o + 512], t1_)

            nc.sync.dma_start(out=dout[0:128, :], in_=out0)
            nc.sync.dma_start(out=dout[128:192, :], in_=out1)
    nc.compile()
    return nc


def _host_prep(x, ln_g, ln_b, w1, b1, dw, bdw, w2, b2, wg, bg):
    f = np.float32
    shared = {}
    W1g = w1 * ln_g[:, None, :]                        # (E, INNER, DIM)
    s1 = -W1g.sum(axis=2)                              # (E, INNER), negated
    c1 = np.einsum('eic,ec->ei', w1, ln_b) + b1        # (E, INNER)
    shared["w1a0"] = np.ascontiguousarray(
        np.transpose(W1g[:, :, 0:128], (0, 2, 1))).astype(BF16)
    w1a1 = np.concatenate([
        np.transpose(W1g[:, :, 128:192], (0, 2, 1)),
        s1[:, None, :], c1[:, None, :]], axis=1)
    shared["w1a1"] = np.ascontiguousarray(w1a1).astype(BF16)
    w2t = np.transpose(w2, (0, 2, 1)).reshape(E, NCHUNK, 128, DIM)
    shared["w2t"] = np.ascontiguousarray(
        np.transpose(w2t, (0, 2, 1, 3)).reshape(E, 128, NCHUNK * DIM)
    ).astype(BF16)
    dwf = dw[:, :, 0]                                  # (E, INNER, 3, 3)
    eye = np.eye(128, dtype=f)
    diag = np.zeros((E, 128, NCHUNK, 9, 128), f)
    for mc in range(NCHUNK):
        for k in range(9):
            dy, dx = k // 3, k % 3
            col = dwf[:, mc * 128:(mc + 1) * 128, dy, dx]
            diag[:, :, mc, k, :] = col[:, :, None] * eye[None, :, :]
    shared["diag"] = diag.astype(BF16).reshape(E, 128, NCHUNK * 9 * 128)
    dwc = np.zeros((E, 128, NCHUNK * 9), f)
    for mc in range(NCHUNK):
        for k in range(9):
            dy, dx = k // 3, k % 3
            dwc[:, :, mc * 9 + k] = dwf[:, mc * 128:(mc + 1) * 128, dy, dx]
    shared["dwc"] = dwc
    shared["bdw"] = np.ascontiguousarray(
        bdw.reshape(E, NCHUNK, 128).transpose(0, 2, 1)).astype(f)
    shared["c2s"] = np.ascontiguousarray(b2).astype(f)
    shared["wg0"] = np.ascontiguousarray(wg.T[0:128]).astype(f)
    shared["wg1"] = np.ascontiguousarray(wg.T[128:192]).astype(f)
    shared["bg"] = bg.reshape(8, 1).astype(f)
    shared["ident"] = np.eye(128, dtype=f)
    shared["ones"] = np.ones((128, 1), f)

    in_maps = []
    xp = np.zeros((B, DIM, H + 2, W), f)
    xp[:, :, 1:H + 1, :] = x
    for core in range(8):
        b, half = core // 2, core % 2
        r0 = half * 32
        xs = xp[b, :, r0:r0 + ROWS, :]
        hm = np.ones((1, ROWS, W), f)
        if half == 0:
            hm[:, 0, :] = 0
        else:
            hm[:, ROWS - 1, :] = 0
        m = dict(shared)
        m["x0"] = np.ascontiguousarray(xs[0:128].reshape(128, NPOS))
        m["x1"] = np.ascontiguousarray(xs[128:192].reshape(64, NPOS))
        m["x0f"] = m["x0"]
        m["x1f"] = m["x1"]
        m["hmask"] = hm.reshape(1, NPOS).astype(BF16)
        in_maps.append(m)
    return in_maps


def _run(inputs, trace=False):
    from concourse.bass_utils import run_bass_kernel_spmd
    if "nc" not in _CACHE:
        _CACHE["nc"] = _build_nc()
    nc = _CACHE["nc"]
    in_maps = _host_prep(**inputs)
    res = run_bass_kernel_spmd(nc, in_maps, core_ids=list(range(8)), trace=trace)
    out = np.empty((B, DIM, H, W), np.float32)
    for core in range(8):
        b, half = core // 2, core % 2
        out[b, :, half * 32:(half + 1) * 32, :] = \
            np.asarray(res.results[core]["out"]).reshape(DIM, 32, W)
    return out, res


def kernel(**inputs) -> np.ndarray:
    inputs = {k: np.asarray(v, dtype=np.float32) for k, v in inputs.items()}
    out, _ = _run(inputs, trace=False)
    return out


# revision 24
# speedup vs baseline: 3.1442x; 3.0937x over previous
"""MoE FeedForward (dense 8-expert, top-2 gate) TRN2 Bass kernel — v3 bf16.

Sharding: 8 shards = (batch b in 0..3) x (H-half in {top, bottom}).
Each NeuronCore computes all 8 experts + gate + top-2 combine for its
32-row spatial slab. Input shards carry a 1-row halo (depthwise conv);
gather on host is pure concatenation.

v3 (vs v1 baseline):
  - Up-projection in bf16 (FWL-eligible weight loads, vs fp32r which
    loads weights slowly) over a bf16 xs (x*inv) + augmented rows
    t1=-mu*inv (weights s1) and hmask (weights c1).
  - Single x load (gate logits + stats read the same f32r x tiles).
  - Unpadded h tiles [128, 34*64]: depthwise edge handling via
    column-restricted out/rhs APs (first op = full-coverage dx=1 tap);
    no pad-column memsets.
  - GELU in 1024-col instructions.
  - fp16 output accumulators (DVE mult by gate weight, Pool add);
    fp16 output DMA, host casts to fp32.
  - Tap engines per chunk tunable: PE diag-matmul / MIX (DVE TS+TT
    tree) / AD (ACT muls + DVE/Pool add tree).
"""
import numpy as np
import ml_dtypes

DIM, MULT, E, TOPK = 192, 4, 8, 2
INNER = DIM * MULT            # 768
B, H, W = 4, 64, 64
EPS = 1e-5
ROWS = 34                     # 32 + 2 halo
NPOS = ROWS * W               # 2176
NOUT = 32 * W                 # 2048
NCHUNK = INNER // 128         # 6

# tap engine per chunk: 'PE' | 'MIX' | 'AD'
TAPS = {0: "PE", 1: "PE", 2: "PE", 3: "MIX", 4: "MIX", 5: "AD"}
# h psum->sbuf copy engine per chunk
HCOPY = {0: "ACT", 1: "DVE", 2: "ACT", 3: "DVE", 4: "ACT", 5: "ACT"}

BF16 = ml_dtypes.bfloat16

UPT = [(0, 512), (512, 512), (1024, 512), (1536, 512), (2048, 128)]

_CACHE = {}


def _build_nc():
    import concourse.bacc as bacc
    import concourse.tile as tile
    import concourse.bass as bass
    from concourse import mybir

    F32 = mybir.dt.float32
    F32R = mybir.dt.float32r
    BF = mybir.dt.bfloat16
    F16 = mybir.dt.float16
    AF = mybir.ActivationFunctionType
    OP = mybir.AluOpType

    nc = bacc.Bacc("TRN2", target_bir_lowering=False)

    # ---- dram tensors ----
    dx0 = nc.dram_tensor("x0", [128, NPOS], F32R, kind="ExternalInput")
    dx1 = nc.dram_tensor("x1", [64, NPOS], F32R, kind="ExternalInput")
    dhm = nc.dram_tensor("hmask", [1, NPOS], BF, kind="ExternalInput")
    dwg0 = nc.dram_tensor("wg0", [128, 8], F32, kind="ExternalInput")
    dwg1 = nc.dram_tensor("wg1", [64, 8], F32, kind="ExternalInput")
    dx0f = nc.dram_tensor("x0f", [128, NPOS], F32, kind="ExternalInput")
    dx1f = nc.dram_tensor("x1f", [64, NPOS], F32, kind="ExternalInput")
    dbg = nc.dram_tensor("bg", [8, 1], F32, kind="ExternalInput")
    dones = nc.dram_tensor("ones", [128, 1], F32R, kind="ExternalInput")
    dw1a0 = nc.dram_tensor("w1a0", [E, 128, INNER], BF, kind="ExternalInput")
    dw1a1 = nc.dram_tensor("w1a1", [E, 66, INNER], BF, kind="ExternalInput")
    dw2t = nc.dram_tensor("w2t", [E, 128, NCHUNK * DIM], BF, kind="ExternalInput")
    ddiag = nc.dram_tensor("diag", [E, 128, NCHUNK * 9 * 128], BF,
                           kind="ExternalInput")
    ddwc = nc.dram_tensor("dwc", [E, 128, NCHUNK * 9], F32, kind="ExternalInput")
    dbdw = nc.dram_tensor("bdw", [E, 128, NCHUNK], F32, kind="ExternalInput")
    dc2 = nc.dram_tensor("c2s", [8, DIM], F32R, kind="ExternalInput")
    dident = nc.dram_tensor("ident", [128, 128], F32, kind="ExternalInput")
    dout = nc.dram_tensor("out", [DIM, NOUT], F32, kind="ExternalOutput")
    dinvs = nc.dram_tensor("invscratch", [1, NPOS], F32, kind="Internal")
    dwalls = nc.dram_tensor("wallscratch", [8, NOUT], BF, kind="Internal")

    NT_ALL = [(0, 512), (512, 512), (1024, 512), (1536, 320), (1856, 320)]
    PE_CH = [mc for mc in range(NCHUNK) if TAPS[mc] == "PE"]

    with tile.TileContext(nc) as tc:
        with tc.tile_pool(name="persist", bufs=1) as pp, \
             tc.tile_pool(name="acc", bufs=1) as accp:
            x0 = pp.tile([128, NPOS], F32R)
            x1 = pp.tile([64, NPOS], F32R)
            xs0 = pp.tile([128, NPOS], BF)
            xs1 = pp.tile([66, NPOS], BF)
            ident = pp.tile([128, 128], F32)
            wg0 = pp.tile([128, 8], F32)
            wg1 = pp.tile([64, 8], F32)
            bg = pp.tile([8, 1], F32)
            ones = pp.tile([128, 1], F32R)
            w_all = pp.tile([8, NOUT], F32R)
            out0 = accp.tile([128, NOUT], F32)
            out1 = accp.tile([64, NOUT], F32)

            nc.sync.dma_start(out=x0, in_=dx0[:, :])
            nc.sync.dma_start(out=x1, in_=dx1[:, :])
            nc.gpsimd.dma_start(out=xs1[65:66, :], in_=dhm[:, :])
            nc.sync.dma_start(out=ident, in_=dident[:, :])
            nc.sync.dma_start(out=wg0, in_=dwg0[:, :])
            nc.sync.dma_start(out=wg1, in_=dwg1[:, :])
            nc.sync.dma_start(out=bg, in_=dbg[:, :])
            nc.sync.dma_start(out=ones, in_=dones[:, :])

            # ---------------- stage 0 ----------------
            with tc.tile_pool(name="s0sb", bufs=2) as s0sb, \
                 tc.tile_pool(name="s0ps", bufs=1, space="PSUM") as s0ps, \
                 tc.tile_pool(name="s0row", bufs=1) as s0row:
                Lsb = s0row.tile([8, NOUT], F32)
                S1row = s0row.tile([1, NPOS], F32)
                S2row = s0row.tile([1, NPOS], F32)
                x0f = s0sb.tile([128, NPOS], F32, tag="x0f", bufs=1)
                x1f = s0sb.tile([64, NPOS], F32, tag="x1f", bufs=1)
                nc.sync.dma_start(out=x0f, in_=dx0f[:, :])
                nc.sync.dma_start(out=x1f, in_=dx1f[:, :])

                for i in range(4):
                    o = 64 + i * 512
                    pl = s0ps.tile([8, 512], F32, tag="pl")
                    nc.tensor.matmul(pl, wg0[:], x0f[:, o:o + 512],
                                     start=True, stop=False)
                    nc.tensor.matmul(pl, wg1[:], x1f[:, o:o + 512],
                                     start=False, stop=True)
                    nc.vector.tensor_scalar(out=Lsb[:, i * 512:(i + 1) * 512],
                                            in0=pl, scalar1=bg[:, :], scalar2=None,
                                            op0=OP.add)

                for (o, n) in NT_ALL:
                    q0 = s0sb.tile([128, 512], F32R, tag="q0")
                    q1 = s0sb.tile([64, 512], F32R, tag="q1")
                    nc.scalar.activation(q0[:, 0:n], x0[:, o:o + n], AF.Square)
                    nc.scalar.activation(q1[:, 0:n], x1[:, o:o + n], AF.Square)
                    psS1 = s0ps.tile([1, 512], F32, tag="psS1")
                    nc.tensor.matmul(psS1[:, 0:n], ones[:], x0[:, o:o + n],
                                     start=True, stop=False)
                    nc.tensor.matmul(psS1[:, 0:n], ones[0:64, :], x1[:, o:o + n],
                                     start=False, stop=True)
                    nc.vector.tensor_copy(S1row[:, o:o + n], psS1[:, 0:n])
                    psS2 = s0ps.tile([1, 512], F32, tag="psS2")
                    nc.tensor.matmul(psS2[:, 0:n], ones[:], q0[:, 0:n],
                                     start=True, stop=False)
                    nc.tensor.matmul(psS2[:, 0:n], ones[0:64, :], q1[:, 0:n],
                                     start=False, stop=True)
                    nc.vector.tensor_copy(S2row[:, o:o + n], psS2[:, 0:n])

                sbeps = s0row.tile([1, 1], F32)
                nc.vector.memset(sbeps, EPS)
                inv = s0row.tile([1, NPOS], F32)
                musq = s0row.tile([1, NPOS], F32)
                nc.scalar.activation(musq, S1row[:, :], AF.Square, scale=1.0 / DIM)
                v1 = s0row.tile([1, NPOS], F32)
                nc.vector.scalar_tensor_tensor(out=v1, in0=S2row[:, :],
                                               scalar=1.0 / DIM, in1=musq,
                                               op0=OP.mult, op1=OP.subtract)
                sd = s0row.tile([1, NPOS], F32)
                nc.scalar.activation(sd, v1, AF.Sqrt, bias=sbeps[:, :], scale=1.0)
                nc.vector.reciprocal_approx_fast(inv, sd)
                nc.vector.scalar_tensor_tensor(out=xs1[64:65, :], in0=S1row[:, :],
                                               scalar=1.0 / DIM, in1=inv,
                                               op0=OP.mult, op1=OP.mult)
                nc.sync.dma_start(out=dinvs[:, :], in_=inv)
                ivap = dinvs[0:1, :]
                inv_src = bass.AP(tensor=ivap.tensor, offset=ivap.offset,
                                  ap=[[0, 128]] + ivap.ap[1:])
                inv_b = s0row.tile([128, NPOS], F32)
                nc.gpsimd.dma_start(out=inv_b, in_=inv_src)
                for (o, n) in NT_ALL:
                    nc.vector.tensor_mul(xs0[:, o:o + n], x0[:, o:o + n],
                                         inv_b[:, o:o + n])
                    nc.vector.tensor_mul(xs1[0:64, o:o + n], x1[:, o:o + n],
                                         inv_b[0:64, o:o + n])

                # ---- top-2 gate in transposed layout ----
                LT = s0row.tile([128, 128], F32)
                for c in range(16):
                    pt = s0ps.tile([128, 8], F32, tag="pt")
                    nc.tensor.transpose(pt, Lsb[:, c * 128:(c + 1) * 128],
                                        ident[0:8, 0:8])
                    nc.vector.tensor_copy(LT[:, c * 8:(c + 1) * 8], pt)
                LTv = LT[:, :].rearrange("p (c e) -> p c e", e=8)
                M1 = s0row.tile([128, 16], F32)
                nc.vector.tensor_reduce(M1, LTv, axis=mybir.AxisListType.X, op=OP.max)

                def bc8(t):
                    a = t[:, :]
                    return bass.AP(tensor=a.tensor, offset=a.offset,
                                   ap=a.ap + [[0, 8]])

                LR = s0row.tile([128, 128], F32)
                nc.vector.tensor_sub(LR[:, :].rearrange("p (c e) -> p c e", e=8),
                                     LTv, bc8(M1))
                EQ = s0row.tile([128, 128], F32)
                nc.vector.tensor_scalar(out=EQ, in0=LR, scalar1=0.0, scalar2=None,
                                        op0=OP.is_equal)
                TMP = s0row.tile([128, 128], F32)
                nc.vector.scalar_tensor_tensor(out=TMP, in0=EQ, scalar=-1e30,
                                               in1=LR, op0=OP.mult, op1=OP.add)
                M2 = s0row.tile([128, 16], F32)
                nc.vector.tensor_reduce(M2, TMP[:, :].rearrange("p (c e) -> p c e", e=8),
                                        axis=mybir.AxisListType.X, op=OP.max)
                EX = s0row.tile([128, 128], F32)
                nc.scalar.activation(EX, LR, AF.Exp)
                ED = s0row.tile([128, 16], F32)
                nc.scalar.activation(ED, M2, AF.Exp)
                DEN = s0row.tile([128, 16], F32)
                nc.vector.tensor_scalar(out=DEN, in0=ED, scalar1=1.0, scalar2=None,
                                        op0=OP.add)
                RC = s0row.tile([128, 16], F32)
                nc.vector.reciprocal_approx_fast(RC, DEN)
                KEEP = s0row.tile([128, 128], F32)
                nc.vector.tensor_tensor(out=KEEP[:, :].rearrange("p (c e) -> p c e", e=8),
                                        in0=LR[:, :].rearrange("p (c e) -> p c e", e=8),
                                        in1=bc8(M2), op=OP.is_ge)
                WT = s0row.tile([128, 128], F32)
                nc.vector.tensor_mul(WT, EX, KEEP)
                nc.vector.tensor_mul(WT[:, :].rearrange("p (c e) -> p c e", e=8),
                                     WT[:, :].rearrange("p (c e) -> p c e", e=8),
                                     bc8(RC))
                for c in range(16):
                    pw = s0ps.tile([8, 128], F32, tag="pw")
                    nc.tensor.transpose(pw, WT[:, c * 8:(c + 1) * 8], ident[:, :])
                    nc.vector.tensor_copy(w_all[:, c * 128:(c + 1) * 128], pw)
                wbf = s0row.tile([8, NOUT], BF)
                nc.vector.tensor_copy(wbf, w_all)
                nc.sync.dma_start(out=dwalls[:, :], in_=wbf)

                c2sb = s0sb.tile([8, DIM], F32R, tag="c2")
                nc.sync.dma_start(out=c2sb, in_=dc2[:, :])
                for i in range(4):
                    o = i * 512
                    pd0 = s0ps.tile([128, 512], F32, tag="pd0i")
                    nc.tensor.matmul(pd0, c2sb[:, 0:128], w_all[:, o:o + 512],
                                     start=True, stop=True)
                    nc.vector.tensor_copy(out0[:, o:o + 512], pd0)
                    pd1 = s0ps.tile([64, 512], F32, tag="pd1i")
                    nc.tensor.matmul(pd1, c2sb[:, 128:192], w_all[:, o:o + 512],
                                     start=True, stop=True)
                    nc.vector.tensor_copy(out1[:, o:o + 512], pd1)

            # ---------------- expert loop ----------------
            with tc.tile_pool(name="wts", bufs=2) as wts, \
                 tc.tile_pool(name="hp", bufs=2) as hp, \
                 tc.tile_pool(name="gw", bufs=2) as gwp, \
                 tc.tile_pool(name="tap", bufs=2) as tapp, \
                 tc.tile_pool(name="cmb", bufs=2) as cmbp, \
                 tc.tile_pool(name="psw", bufs=2, space="PSUM") as psw, \
                 tc.tile_pool(name="psd0", bufs=2, space="PSUM") as psd0p, \
                 tc.tile_pool(name="psd1", bufs=2, space="PSUM") as psd1p:
                for e in range(E):
                    W1A0 = wts.tile([128, INNER], BF, tag="w1a0")
                    W1A1 = wts.tile([66, INNER], BF, tag="w1a1")
                    W2T = wts.tile([128, NCHUNK * DIM], BF, tag="w2t")
                    BDW = wts.tile([128, NCHUNK], F32, tag="bdw")
                    W_B = wts.tile([128, NOUT], BF, tag="wb")
                    nc.sync.dma_start(out=W1A0, in_=dw1a0[e, :, :])
                    nc.sync.dma_start(out=W1A1, in_=dw1a1[e, :, :])
                    nc.sync.dma_start(out=W2T, in_=dw2t[e, :, :])
                    nc.sync.dma_start(out=BDW, in_=dbdw[e, :, :])
                    if PE_CH:
                        DIAG = wts.tile([128, len(PE_CH) * 9 * 128], BF,
                                        tag="diag")
                        for ci, mc in enumerate(PE_CH):
                            nc.sync.dma_start(
                                out=DIAG[:, ci * 9 * 128:(ci + 1) * 9 * 128],
                                in_=ddiag[e, :, mc * 9 * 128:(mc + 1) * 9 * 128])
                    if len(PE_CH) < NCHUNK:
                        DWC = wts.tile([128, NCHUNK * 9], F32, tag="dwc")
                        nc.sync.dma_start(out=DWC, in_=ddwc[e, :, :])
                    wsrc = dwalls[e:e + 1, :]
                    nc.gpsimd.dma_start(
                        out=W_B,
                        in_=bass.AP(tensor=wsrc.tensor, offset=wsrc.offset,
                                    ap=[[0, 128]] + wsrc.ap[1:]))

                    PW = W + 2
                    hpt = []
                    for mc in range(NCHUNK):
                        if TAPS[mc] == "PE":
                            ht = hp.tile([128, NPOS], BF, tag=f"h{mc}")
                        else:
                            ht = hp.tile([128, ROWS * PW], BF, tag=f"h{mc}")
                            hview = ht[:, :].rearrange("p (r c) -> p r c", c=PW)
                            nc.gpsimd.memset(hview[:, :, 0:1], 0.0)
                            nc.gpsimd.memset(hview[:, :, PW - 1:PW], 0.0)
                        hpt.append(ht)
                    g8 = gwp.tile([128, NCHUNK * NOUT], BF, tag="g8")

                    # ---- up-projection + depthwise taps + gelu, interleaved ----
                    # tap k at (dy,dx)=(k//3,k%3): out(y,x)+=dw_k*h(y+dy-1,x+dx-1)
                    # out rows 1..33; width edges via restricted columns.
                    def colspec(dx):
                        if dx == 0:
                            return W - 1, 1      # ncol, out col base
                        if dx == 1:
                            return W, 0
                        return W - 1, 0

                    for mc in (3, 4, 5, 0, 1, 2):
                        for (o, n) in UPT:
                            ph = psw.tile([128, 512], F32, tag="ph")
                            nc.tensor.matmul(ph[:, 0:n],
                                             W1A0[:, mc * 128:(mc + 1) * 128],
                                             xs0[:, o:o + n],
                                             start=True, stop=False)
                            nc.tensor.matmul(ph[:, 0:n],
                                             W1A1[:, mc * 128:(mc + 1) * 128],
                                             xs1[:, o:o + n],
                                             start=False, stop=True)
                            if TAPS[mc] == "PE":
                                dst = hpt[mc][:, o:o + n]
                            else:
                                r0c = o // W
                                nrc = n // W
                                dst = hpt[mc][:, :].rearrange(
                                    "p (r c) -> p r c", c=PW)[
                                    :, r0c:r0c + nrc, 1:W + 1]
                            if HCOPY[mc] == "ACT":
                                nc.scalar.copy(dst, ph[:, 0:n])
                            else:
                                nc.vector.tensor_copy(dst, ph[:, 0:n])

                    for mc in range(NCHUNK):
                        hap = hpt[mc][:, :]
                        eng = TAPS[mc]
                        if eng == "PE":
                            for d in range(4):
                                pt = psw.tile([128, 512], F32, tag="ptap")
                                ptv = pt[:, :].rearrange("p (r c) -> p r c", c=W)
                                r0 = 1 + 8 * d
                                order = [1, 0, 2, 3, 4, 5, 6, 7, 8]
                                for si, k in enumerate(order):
                                    dy, dx = k // 3, k % 3
                                    ncol, ob = colspec(dx)
                                    ci = PE_CH.index(mc)
                                    dg = DIAG[:, (ci * 9 + k) * 128:
                                              (ci * 9 + k + 1) * 128]
                                    base = hap.offset + (r0 + dy - 1) * W \
                                        + (ob + dx - 1)
                                    rhs = bass.AP(
                                        tensor=hap.tensor, offset=base,
                                        ap=[hap.ap[0], [W, 8], [1, ncol]])
                                    nc.tensor.matmul(
                                        ptv[:, :, ob:ob + ncol], dg, rhs,
                                        start=(si == 0), stop=(si == 8))
                                nc.scalar.activation(
                                    g8[:, mc * NOUT + d * 512:
                                       mc * NOUT + (d + 1) * 512],
                                    pt, AF.Gelu, bias=BDW[:, mc:mc + 1],
                                    scale=1.0)
                        else:
                            hv66 = hap.rearrange("p (r c) -> p r c", c=PW)
                            for ti in range(2):
                                r0 = 1 + 16 * ti

                                def sc(k):
                                    return DWC[:, mc * 9 + k:mc * 9 + k + 1]

                                def view(k):
                                    dy, dx = k // 3, k % 3
                                    return hv66[:, r0 + dy - 1:r0 + dy + 15,
                                                dx:dx + W]

                                acc = tapp.tile([128, 1024], BF, tag="ta")
                                accv = acc[:, :].rearrange("p (r c) -> p r c", c=W)
                                hb = tapp.tile([128, 1024], BF, tag="tb")
                                hbv = hb[:, :].rearrange("p (r c) -> p r c", c=W)
                                if eng in ("MIX", "MIXP"):
                                    nc.vector.tensor_scalar(
                                        out=accv, in0=view(0), scalar1=sc(0),
                                        scalar2=None, op0=OP.mult)
                                    nc.vector.tensor_scalar(
                                        out=hbv, in0=view(4), scalar1=sc(4),
                                        scalar2=None, op0=OP.mult)
                                    for k in (1, 2, 3, 5, 6, 7, 8):
                                        tm = tapp.tile([128, 1024], BF, tag="tm")
                                        tmv = tm[:, :].rearrange(
                                            "p (r c) -> p r c", c=W)
                                        nc.vector.tensor_scalar(
                                            out=tmv, in0=view(k), scalar1=sc(k),
                                            scalar2=None, op0=OP.mult)
                                        dst = acc if k < 4 else hb
                                        nc.vector.tensor_add(dst, dst, tm)
                                    nc.vector.tensor_add(acc, acc, hb)
                                else:  # AD: ACT muls; adds DVE + Pool trees
                                    nc.scalar.mul(accv, view(0), sc(0))
                                    nc.scalar.mul(hbv, view(4), sc(4))
                                    for k in (1, 2, 3, 5, 6, 7, 8):
                                        tm = tapp.tile([128, 1024], BF, tag="tm")
                                        tmv = tm[:, :].rearrange(
                                            "p (r c) -> p r c", c=W)
                                        nc.scalar.mul(tmv, view(k), sc(k))
                                        if k < 4:
                                            nc.vector.tensor_add(acc, acc, tm)
                                        else:
                                            nc.vector.tensor_add(hb, hb, tm)
                                    nc.vector.tensor_add(acc, acc, hb)
                                nc.scalar.activation(
                                    g8[:, mc * NOUT + ti * 1024:
                                       mc * NOUT + (ti + 1) * 1024],
                                    acc, AF.Gelu, bias=BDW[:, mc:mc + 1],
                                    scale=1.0)

                    # ---- down-projection + combine, per dtile ----
                    for dt in range(4):
                        o = dt * 512
                        pd0 = psd0p.tile([128, 512], F32, tag="pd0")
                        pd1 = psd1p.tile([64, 512], F32, tag="pd1")
                        for kc in range(NCHUNK):
                            nc.tensor.matmul(pd0, W2T[:, kc * DIM:kc * DIM + 128],
                                             g8[:, kc * NOUT + o:kc * NOUT + o + 512],
                                             start=(kc == 0),
                                             stop=(kc == NCHUNK - 1))
                        for kc in range(NCHUNK):
                            nc.tensor.matmul(pd1,
                                             W2T[:, kc * DIM + 128:(kc + 1) * DIM],
                                             g8[:, kc * NOUT + o:kc * NOUT + o + 512],
                                             start=(kc == 0),
                                             stop=(kc == NCHUNK - 1))
                        t0 = cmbp.tile([128, 512], F32, tag="t0")
                        t1_ = cmbp.tile([64, 512], F32, tag="t1")
                        nc.vector.tensor_mul(t0, pd0, W_B[:, o:o + 512])
                        nc.vector.tensor_mul(t1_, pd1, W_B[0:64, o:o + 512])
                        nc.gpsimd.tensor_add(out0[:, o:o + 512],
                                             out0[:, o:o + 512], t0)
                        nc.gpsimd.tensor_add(out1[:, o:o + 512],
                                             out1[:, o:o + 512], t1_)

            nc.sync.dma_start(out=dout[0:128, :], in_=out0)
            nc.sync.dma_start(out=dout[128:192, :], in_=out1)
    nc.compile()
    return nc


def _host_prep(x, ln_g, ln_b, w1, b1, dw, bdw, w2, b2, wg, bg):
    f = np.float32
    shared = {}
    W1g = w1 * ln_g[:, None, :]                        # (E, INNER, DIM)
    s1 = -W1g.sum(axis=2)                              # (E, INNER), negated
    c1 = np.einsum('eic,ec->ei', w1, ln_b) + b1        # (E, INNER)
    shared["w1a0"] = np.ascontiguousarray(
        np.transpose(W1g[:, :, 0:128], (0, 2, 1))).astype(BF16)
    w1a1 = np.concatenate([
        np.transpose(W1g[:, :, 128:192], (0, 2, 1)),
        s1[:, None, :], c1[:, None, :]], axis=1)
    shared["w1a1"] = np.ascontiguousarray(w1a1).astype(BF16)
    w2t = np.transpose(w2, (0, 2, 1)).reshape(E, NCHUNK, 128, DIM)
    shared["w2t"] = np.ascontiguousarray(
        np.transpose(w2t, (0, 2, 1, 3)).reshape(E, 128, NCHUNK * DIM)
    ).astype(BF16)
    dwf = dw[:, :, 0]                                  # (E, INNER, 3, 3)
    eye = np.eye(128, dtype=f)
    diag = np.zeros((E, 128, NCHUNK, 9, 128), f)
    for mc in range(NCHUNK):
        for k in range(9):
            dy, dx = k // 3, k % 3
            col = dwf[:, mc * 128:(mc + 1) * 128, dy, dx]
            diag[:, :, mc, k, :] = col[:, :, None] * eye[None, :, :]
    shared["diag"] = diag.astype(BF16).reshape(E, 128, NCHUNK * 9 * 128)
    dwc = np.zeros((E, 128, NCHUNK * 9), f)
    for mc in range(NCHUNK):
        for k in range(9):
            dy, dx = k // 3, k % 3
            dwc[:, :, mc * 9 + k] = dwf[:, mc * 128:(mc + 1) * 128, dy, dx]
    shared["dwc"] = dwc
    shared["bdw"] = np.ascontiguousarray(
        bdw.reshape(E, NCHUNK, 128).transpose(0, 2, 1)).astype(f)
    shared["c2s"] = np.ascontiguousarray(b2).astype(f)
    shared["wg0"] = np.ascontiguousarray(wg.T[0:128]).astype(f)
    shared["wg1"] = np.ascontiguousarray(wg.T[128:192]).astype(f)
    shared["bg"] = bg.reshape(8, 1).astype(f)
    shared["ident"] = np.eye(128, dtype=f)
    shared["ones"] = np.ones((128, 1), f)

    in_maps = []
    xp = np.zeros((B, DIM, H + 2, W), f)
    xp[:, :, 1:H + 1, :] = x
    for core in range(8):
        b, half = core // 2, core % 2
        r0 = half * 32
        xs = xp[b, :, r0:r0 + ROWS, :]
        hm = np.ones((1, ROWS, W), f)
        if half == 0:
            hm[:, 0, :] = 0
        else:
            hm[:, ROWS - 1, :] = 0
        m = dict(shared)
        m["x0"] = np.ascontiguousarray(xs[0:128].reshape(128, NPOS))
        m["x1"] = np.ascontiguousarray(xs[128:192].reshape(64, NPOS))
        m["x0f"] = m["x0"]
        m["x1f"] = m["x1"]
        m["hmask"] = hm.reshape(1, NPOS).astype(BF16)
        in_maps.append(m)
    return in_maps


def _run(inputs, trace=False):
    from concourse.bass_utils import run_bass_kernel_spmd
    if "nc" not in _CACHE:
        _CACHE["nc"] = _build_nc()
    nc = _CACHE["nc"]
    in_maps = _host_prep(**inputs)
    res = run_bass_kernel_spmd(nc, in_maps, core_ids=list(range(8)), trace=trace)
    out = np.empty((B, DIM, H, W), np.float32)
    for core in range(8):
        b, half = core // 2, core % 2
        out[b, :, half * 32:(half + 1) * 32, :] = \
            np.asarray(res.results[core]["out"]).reshape(DIM, 32, W)
    return out, res


def kernel(**inputs) -> np.ndarray:
    inputs = {k: np.asarray(v, dtype=np.float32) for k, v in inputs.items()}
    out, _ = _run(inputs, trace=False)
    return out
